# revision 6
# baseline (speedup 1.0000x reference)
"""Cross-modal attention (CMAttention) Trainium2 kernel.

Strategy: 8-way SPMD over (batch=4) x (modality=2). After the reference's
concat([q_x, q_a]) and 8-head split with head_dim=128, heads 0-3 depend only
on modality x and heads 4-7 only on modality a.  Each core therefore owns one
(batch, modality) pair end-to-end with zero communication:
  QKV projection (bf16 matmul) -> LayerNorm on q,k (bn_stats) -> RoPE
  (table multiplies) -> per-head DMA-transpose of q,k to [d, tok] ->
  scores^T matmul -> exp on ScalarE (scale folded) -> attn @ [v | 1]
  (ones column yields the softmax denominator for free) -> normalize.

Schedule notes (v2):
  - xT chunks load on the sync HWDGE ring, W chunks on the scalar HWDGE
    ring in parallel, so the first matmul starts ~2us in (the GpSimd ring
    opens with ~8us of DRAINs and is kept off the critical path).
  - rstd = exp(-0.5*ln(var+eps)) keeps the natural_log_exp activation
    table resident for the whole kernel (no table switch before stage B's
    exp); the LN apply and the v PSUM->SBUF copy also run on ScalarE to
    keep VectorE at ~3.5us/tile.
  - rope writes m1 = u*cos and m2 = swap(u)*sin separately; the add
    happens in the SWDGE accumulating store into r_dram (GpSimd ring),
    freeing VectorE of the add.
  - q/k transposes are issued per 512-token half as soon as that half is
    spilled, so the stage A->B boundary only waits on head 0's half-1
    transposes (~1.5us) instead of the full 23us spill+transpose chain.
  - outputs DMA per (head, qc) 128x128 block right after normalization,
    so the tail after the last AV matmul is ~1us.
"""

import os
import sys

for _p in ("/opt/trn_rl_repo", os.path.expanduser("~/.axon_site/_ro/trn_rl_repo")):
    if os.path.isdir(_p) and _p not in sys.path:
        sys.path.append(_p)

from contextlib import ExitStack

import ml_dtypes
import numpy as np

import concourse.bacc as bacc
import concourse.bass as bass
import concourse.mybir as mybir
import concourse.tile as tile
from concourse.bass_utils import run_bass_kernel_spmd

if os.environ.get("K_LDWOPT"):
    import concourse.bass_utils as _bu

    _orig_run_command = _bu.run_command

    def _patched_run_command(argv, **kw):
        argv = [
            "--enable-ldw-opt=true" if a == "--enable-ldw-opt=false" else a
            for a in argv
        ]
        return _orig_run_command(argv, **kw)

    _bu.run_command = _patched_run_command

BF16 = mybir.dt.float16
F32 = mybir.dt.float32
NPBF16 = np.float16

DIM = 512          # per-modality feature dim
N_TOK = 1024       # sequence length
NH = 4             # heads handled per core (one modality's heads)
D = 128            # head dim
NT = 8             # token tiles of 128
EPS = 1e-5
SCALE = 1.0 / float(np.sqrt(D))
VW = 132           # per-head v block width: 128 d + 1 ones + 3 pad
AF = mybir.ActivationFunctionType


def build_module(trivial: bool):
    """Build the per-core Bass program.  trivial=True assumes all LN gains are
    exactly 1 and biases exactly 0 (folded tables are plain cos/sin and the
    additive rope term vanishes); trivial=False uses full-width tables with
    g folded in and T3 pre-added into r_dram via a DRAM->DRAM copy."""
    nc = bacc.Bacc("TRN2", target_bir_lowering=False, debug=False, num_devices=8)

    xT = nc.dram_tensor("xT", [DIM, N_TOK], BF16, kind="ExternalInput")
    W = nc.dram_tensor("W", [DIM, 3 * DIM], BF16, kind="ExternalInput")
    if trivial:
        T1 = nc.dram_tensor("T1", [N_TOK, 64], BF16, kind="ExternalInput")
        T2N = nc.dram_tensor("T2N", [N_TOK, 64], BF16, kind="ExternalInput")
        T2P = nc.dram_tensor("T2P", [N_TOK, 64], BF16, kind="ExternalInput")
    else:
        T1 = nc.dram_tensor("T1", [N_TOK, 1024], BF16, kind="ExternalInput")
        T2 = nc.dram_tensor("T2", [N_TOK, 1024], BF16, kind="ExternalInput")
        T3 = nc.dram_tensor("T3", [N_TOK, 1024], BF16, kind="ExternalInput")
    out_d = nc.dram_tensor("out", [N_TOK, DIM], F32, kind="ExternalOutput")

    with tile.TileContext(nc) as tc, ExitStack() as ctx:
        consts = ctx.enter_context(tc.tile_pool(name="consts", bufs=1))
        small = ctx.enter_context(tc.tile_pool(name="small", bufs=4))
        upool = ctx.enter_context(tc.tile_pool(name="upool", bufs=2))
        rpool = ctx.enter_context(tc.tile_pool(name="rpool", bufs=2))
        epool = ctx.enter_context(tc.tile_pool(name="epool", bufs=2))
        dpool = ctx.enter_context(tc.tile_pool(name="dpool", bufs=1, space="DRAM"))
        psum_qk = ctx.enter_context(tc.tile_pool(name="psqk", bufs=3, space="PSUM"))
        psum_v = ctx.enter_context(tc.tile_pool(name="psv", bufs=2, space="PSUM"))

        # ---- input loads spread over three rings (sync/scalar HWDGE +
        # gpsimd SWDGE) so the first matmul's coarse ring-ordinal waits
        # cover as little as possible ----
        xr = xT.ap().rearrange("(a b) c -> b a c", b=128)
        wr = W.ap().rearrange("(a b) c -> b a c", b=128)
        xT_k, W_k = [], []
        for kc in range(4):
            xt = consts.tile([128, N_TOK], BF16, name=f"xT{kc}", tag=f"xT{kc}")
            xT_k.append(xt)
            wt = consts.tile([128, 3 * DIM], BF16, name=f"W{kc}", tag=f"W{kc}")
            W_k.append(wt)
        nc.sync.dma_start(out=xT_k[0], in_=xr[:, 0])
        nc.scalar.dma_start(out=W_k[0], in_=wr[:, 0])
        nc.sync.dma_start(out=xT_k[1], in_=xr[:, 1])
        nc.scalar.dma_start(out=W_k[1], in_=wr[:, 1])
        nc.gpsimd.dma_start(out=xT_k[2], in_=xr[:, 2])
        nc.gpsimd.dma_start(out=W_k[2], in_=wr[:, 2])
        nc.gpsimd.dma_start(out=xT_k[3], in_=xr[:, 3])
        nc.gpsimd.dma_start(out=W_k[3], in_=wr[:, 3])
        if trivial:
            cos_sb = consts.tile([128, NT, 64], BF16, tag="cos")
            nc.sync.dma_start(
                out=cos_sb, in_=T1.ap().rearrange("(a b) c -> b a c", b=128)
            )
            sinN_sb = consts.tile([128, NT, 64], BF16, tag="sinN")
            nc.sync.dma_start(
                out=sinN_sb, in_=T2N.ap().rearrange("(a b) c -> b a c", b=128)
            )
            sinP_sb = consts.tile([128, NT, 64], BF16, tag="sinP")
            nc.sync.dma_start(
                out=sinP_sb, in_=T2P.ap().rearrange("(a b) c -> b a c", b=128)
            )
        else:
            T1_sb = consts.tile([128, NT, 1024], BF16, tag="T1")
            nc.sync.dma_start(
                out=T1_sb, in_=T1.ap().rearrange("(a b) c -> b a c", b=128)
            )
            T2_sb = consts.tile([128, NT, 1024], BF16, tag="T2")
            nc.scalar.dma_start(
                out=T2_sb, in_=T2.ap().rearrange("(a b) c -> b a c", b=128)
            )
        eps_sb = consts.tile([128, 1], F32, tag="eps")
        nc.vector.memset(eps_sb, EPS)

        v_sb = consts.tile([128, NT, NH, VW], BF16, tag="v")
        nc.vector.memset(v_sb[:, :, :, 128:129], 1.0)

        qkT_sb = [
            [
                consts.tile(
                    [128, N_TOK], BF16, name=f"qkT{s}{h}", tag=f"qkT{s}{h}"
                )
                for h in range(NH)
            ]
            for s in range(2)
        ]
        out_sb = consts.tile([128, NT, DIM], F32, tag="osb")
        r_dram = dpool.tile([N_TOK, 2 * DIM], BF16, name="r_dram", tag="r_dram")
        if not trivial:
            # pre-add the additive rope/bias term; m1/m2 then accumulate on top
            nc.gpsimd.dma_start(out=r_dram, in_=T3.ap())

        def bcast(ap2d, dims):
            """[128, 64] AP -> [128, *dims, 64] with stride-0 broadcast dims."""
            p, last = ap2d.ap[0], ap2d.ap[-1]
            return bass.AP(
                tensor=ap2d.tensor,
                offset=ap2d.offset,
                ap=[p] + [[0, d] for d in dims] + [last],
            )

        def half(ap, i):
            return ap.rearrange("p (b half j) -> p b half j", half=2, j=64)[
                :, :, i, :
            ]

        # ---------------- stage A: one 128-token tile ----------------
        u_of = {}

        def stage_a1(t):
            qkv_ps = psum_qk.tile([128, 2 * DIM], F32, tag="qk", name="qkv_ps")
            v_ps = psum_v.tile([128, DIM], F32, tag="v", name="v_ps")
            for kc in range(4):
                for j in range(2):
                    nc.tensor.matmul(
                        qkv_ps[:, j * 512 : (j + 1) * 512],
                        lhsT=xT_k[kc][:, t * 128 : (t + 1) * 128],
                        rhs=W_k[kc][:, j * 512 : (j + 1) * 512],
                        start=(kc == 0),
                        stop=(kc == 3),
                    )
                nc.tensor.matmul(
                    v_ps,
                    lhsT=xT_k[kc][:, t * 128 : (t + 1) * 128],
                    rhs=W_k[kc][:, 1024:1536],
                    start=(kc == 0),
                    stop=(kc == 3),
                )

            u = upool.tile([128, 2 * DIM], BF16, tag="u", name="u")
            for s in range(2):
                st = small.tile([128, 6], F32, tag=f"st{s}", name="st")
                nc.vector.bn_stats(out=st, in_=qkv_ps[:, s * 512 : (s + 1) * 512])
                mv = small.tile([128, 2], F32, tag=f"mv{s}", name="mv")
                nc.vector.bn_aggr(out=mv, in_=st)
                sd = small.tile([128, 1], F32, tag=f"sd{s}", name="sd")
                nc.scalar.activation(sd, mv[:, 1:2], AF.Sqrt, bias=eps_sb)
                rstd = small.tile([128, 1], F32, tag=f"rs{s}", name="rstd")
                nc.vector.reciprocal(rstd, sd)
                nmr = small.tile([128, 1], F32, tag=f"nmr{s}", name="nmr")
                nc.vector.scalar_tensor_tensor(
                    out=nmr,
                    in0=mv[:, 0:1],
                    scalar=-1.0,
                    in1=rstd,
                    op0=mybir.AluOpType.mult,
                    op1=mybir.AluOpType.mult,
                )
                # u = q * rstd + (-mu * rstd), on ScalarE (frees VectorE)
                nc.scalar.activation(
                    out=u[:, s * 512 : (s + 1) * 512],
                    in_=qkv_ps[:, s * 512 : (s + 1) * 512],
                    func=AF.Identity,
                    scale=rstd,
                    bias=nmr,
                )

            # v (raw) into augmented per-head layout, on ScalarE
            nc.scalar.activation(
                out=v_sb[:, t, :, 0:128],
                in_=v_ps.rearrange("p (h d) -> p h d", h=NH),
                func=AF.Copy,
            )

            u_of[t] = u

        def stage_a2(t):
            u = u_of.pop(t)
            # rope: r = u * T1 + swap_half(u) * T2 (+ T3); the halves are
            # combined by accumulating SWDGE stores into r_dram (GpSimd is
            # otherwise idle and this frees VectorE of the add)
            m1 = rpool.tile([128, 2 * DIM], BF16, tag="m1", name="m1")
            m2 = rpool.tile([128, 2 * DIM], BF16, tag="m2", name="m2")
            if trivial:
                nc.vector.tensor_mul(
                    m1.rearrange("p (b j) -> p b j", j=64),
                    u.rearrange("p (b j) -> p b j", j=64),
                    bcast(cos_sb[:, t], (16,)),
                )
                nc.vector.tensor_mul(
                    half(m2, 0), half(u, 1), bcast(sinN_sb[:, t], (8,))
                )
                nc.vector.tensor_mul(
                    half(m2, 1), half(u, 0), bcast(sinP_sb[:, t], (8,))
                )
            else:
                nc.vector.tensor_mul(m1, u, T1_sb[:, t])
                nc.vector.tensor_mul(half(m2, 0), half(u, 1), half(T2_sb[:, t], 0))
                nc.vector.tensor_mul(half(m2, 1), half(u, 0), half(T2_sb[:, t], 1))
            rows = slice(t * 128, (t + 1) * 128)
            m1_op = (
                mybir.AluOpType.bypass if trivial else mybir.AluOpType.add
            )
            nc.gpsimd.dma_start(out=r_dram[rows, :], in_=m1, accum_op=m1_op)
            nc.gpsimd.dma_start(
                out=r_dram[rows, :], in_=m2, accum_op=mybir.AluOpType.add
            )

        def emit_transposes(H):
            rows = slice(H * 512, (H + 1) * 512)
            for h in range(NH):
                for s in range(2):
                    blk = (s * NH + h) * 128
                    nc.sync.dma_start(
                        out=qkT_sb[s][h][:, rows],
                        in_=r_dram[rows, blk : blk + 128],
                        transpose=True,
                    )

        # ---------------- stage B pieces ----------------
        # exp tiles: ets[h][kc] = exp(scores^T) [128 k, 1024 q] bf16
        ets = [[None] * NT for _ in range(NH)]

        def scores_kc(h, kc):
            qT, kT = qkT_sb[0][h], qkT_sb[1][h]
            sc_ps = psum_qk.tile([128, 2 * DIM], F32, tag="qk", name="sc_ps")
            for qh in range(2):
                nc.tensor.matmul(
                    sc_ps[:, qh * 512 : (qh + 1) * 512],
                    lhsT=kT[:, kc * 128 : (kc + 1) * 128],
                    rhs=qT[:, qh * 512 : (qh + 1) * 512],
                    start=True,
                    stop=True,
                )
            et = epool.tile(
                [128, N_TOK], BF16, tag=f"exp{h % 2}_{kc}",
                name=f"exp{h % 2}_{kc}", bufs=1,
            )
            nc.scalar.activation(
                out=et,
                in_=sc_ps[:, 0:N_TOK],
                func=AF.Exp,
                scale=SCALE,
            )
            ets[h][kc] = et

        def emit_scores(h):
            for kc in range(NT):
                scores_kc(h, kc)

        def emit_av(h):
            for qc in range(NT):
                av = psum_v.tile([128, VW], F32, tag="v", name="av")
                for kc in range(NT):
                    et = ets[h][kc]
                    nc.tensor.matmul(
                        av[:, 0:129],
                        lhsT=et[:, qc * 128 : (qc + 1) * 128],
                        rhs=v_sb[:, kc, h, 0:129],
                        start=(kc == 0),
                        stop=(kc == NT - 1),
                    )
                rcp = small.tile([128, 1], F32, tag="rcp", name="rcp")
                nc.vector.reciprocal(rcp, av[:, 128:129])
                dst = out_sb[:, qc, h * 128 : (h + 1) * 128]
                nc.vector.tensor_scalar_mul(dst, av[:, 0:128], rcp)
                # per-(head, qc) output block on the SWDGE ring (idle in
                # stage B); tail after the last AV ~1us
                nc.gpsimd.dma_start(
                    out=out_d.ap()[
                        qc * 128 : (qc + 1) * 128, h * 128 : (h + 1) * 128
                    ],
                    in_=dst,
                )

        # ---------------- emission schedule ----------------
        stage_a1(0)
        for t in range(1, NT):
            stage_a1(t)
            stage_a2(t - 1)
            if t == 4:
                emit_transposes(0)
        stage_a2(NT - 1)
        emit_transposes(1)

        emit_scores(0)
        for h in range(NH):
            if h + 1 < NH:
                emit_scores(h + 1)
            emit_av(h)

    nc.compile()
    return nc


def _rope_tables():
    inv_freq = 1.0 / (10000.0 ** (np.arange(0, D, 2, dtype=np.float32) / D))
    freqs = np.arange(N_TOK, dtype=np.float32)[:, None] * inv_freq[None, :]  # [n, 64]
    return np.cos(freqs), np.sin(freqs)


def _full_tables(g_q, b_q, g_k, b_k):
    """T1/T2/T3 [N_TOK, 1024] with LN gain/bias folded into the rope tables.
    Feature index layout matches u: (s, h, half, j)."""
    cos64, sin64 = _rope_tables()
    T1 = np.empty((N_TOK, 1024), np.float32)
    T2 = np.empty((N_TOK, 1024), np.float32)
    T3 = np.empty((N_TOK, 1024), np.float32)
    for s, (g, b) in enumerate(((g_q, b_q), (g_k, b_k))):
        g = g.reshape(NH, 2, 64)
        b = b.reshape(NH, 2, 64)
        for h in range(NH):
            base = s * 512 + h * 128
            lo, hi = slice(base, base + 64), slice(base + 64, base + 128)
            T1[:, lo] = g[h, 0] * cos64
            T1[:, hi] = g[h, 1] * cos64
            T2[:, lo] = -g[h, 1] * sin64
            T2[:, hi] = g[h, 0] * sin64
            T3[:, lo] = b[h, 0] * cos64 - b[h, 1] * sin64
            T3[:, hi] = b[h, 1] * cos64 + b[h, 0] * sin64
    return T1, T2, T3


def make_in_maps(x, a, Wqkv_x, Wqkv_a, g_qx, b_qx, g_kx, b_kx, g_qa, b_qa, g_ka, b_ka):
    """Returns (trivial, in_maps) for the 8 cores: core c = (batch c//2, modality c%2)."""
    x, a = np.asarray(x), np.asarray(a)
    Ws = (np.asarray(Wqkv_x), np.asarray(Wqkv_a))
    gb = (
        (np.asarray(g_qx), np.asarray(b_qx), np.asarray(g_kx), np.asarray(b_kx)),
        (np.asarray(g_qa), np.asarray(b_qa), np.asarray(g_ka), np.asarray(b_ka)),
    )
    trivial = all(
        np.all(g == 1.0) and np.all(b == 0.0)
        for (gq, bq, gk, bk) in gb
        for g, b in ((gq, bq), (gk, bk))
    )
    cos64, sin64 = _rope_tables()
    in_maps = []
    for c in range(8):
        i, m = c // 2, c % 2
        src = x[i] if m == 0 else a[i]
        im = {
            "xT": np.ascontiguousarray(src.T).astype(NPBF16),
            "W": Ws[m].astype(NPBF16),
        }
        if trivial:
            im["T1"] = cos64.astype(NPBF16)
            im["T2N"] = (-sin64).astype(NPBF16)
            im["T2P"] = sin64.astype(NPBF16)
        else:
            gq, bq, gk, bk = gb[m]
            T1, T2, T3 = _full_tables(gq, bq, gk, bk)
            im["T1"] = T1.astype(NPBF16)
            im["T2"] = T2.astype(NPBF16)
            im["T3"] = T3.astype(NPBF16)
        in_maps.append(im)
    return trivial, in_maps


_module_cache: dict[bool, object] = {}


def _get_module(trivial: bool):
    if trivial not in _module_cache:
        _module_cache[trivial] = build_module(trivial)
    return _module_cache[trivial]


def kernel(**inputs) -> np.ndarray:
    trivial, in_maps = make_in_maps(**inputs)
    nc = _get_module(trivial)
    res = run_bass_kernel_spmd(nc, in_maps, core_ids=list(range(8)))
    out = np.empty((4, N_TOK, 2 * DIM), np.float32)
    for c in range(8):
        i, m = c // 2, c % 2
        out[i, :, m * 512 : (m + 1) * 512] = res.results[c]["out"]
    return out


# revision 12
# speedup vs baseline: 1.0549x; 1.0549x over previous
"""Cross-modal attention (CMAttention) Trainium2 kernel.

Strategy: 8-way SPMD over (batch=4) x (modality=2). After the reference's
concat([q_x, q_a]) and 8-head split with head_dim=128, heads 0-3 depend only
on modality x and heads 4-7 only on modality a.  Each core therefore owns one
(batch, modality) pair end-to-end with zero communication:
  QKV projection (bf16 matmul) -> LayerNorm on q,k (bn_stats) -> RoPE
  (table multiplies) -> per-head DMA-transpose of q,k to [d, tok] ->
  scores^T matmul -> exp on ScalarE (scale folded) -> attn @ [v | 1]
  (ones column yields the softmax denominator for free) -> normalize.

Schedule notes (v2):
  - xT chunks load on the sync HWDGE ring, W chunks on the scalar HWDGE
    ring in parallel, so the first matmul starts ~2us in (the GpSimd ring
    opens with ~8us of DRAINs and is kept off the critical path).
  - rstd = exp(-0.5*ln(var+eps)) keeps the natural_log_exp activation
    table resident for the whole kernel (no table switch before stage B's
    exp); the LN apply and the v PSUM->SBUF copy also run on ScalarE to
    keep VectorE at ~3.5us/tile.
  - rope writes m1 = u*cos and m2 = swap(u)*sin separately; the add
    happens in the SWDGE accumulating store into r_dram (GpSimd ring),
    freeing VectorE of the add.
  - q/k transposes are issued per 512-token half as soon as that half is
    spilled, so the stage A->B boundary only waits on head 0's half-1
    transposes (~1.5us) instead of the full 23us spill+transpose chain.
  - outputs DMA per (head, qc) 128x128 block right after normalization,
    so the tail after the last AV matmul is ~1us.
"""

import os
import sys

for _p in ("/opt/trn_rl_repo", os.path.expanduser("~/.axon_site/_ro/trn_rl_repo")):
    if os.path.isdir(_p) and _p not in sys.path:
        sys.path.append(_p)

from contextlib import ExitStack

import ml_dtypes
import numpy as np

import concourse.bacc as bacc
import concourse.bass as bass
import concourse.mybir as mybir
import concourse.tile as tile
from concourse.bass_utils import run_bass_kernel_spmd

if os.environ.get("K_LDWOPT"):
    import concourse.bass_utils as _bu

    _orig_run_command = _bu.run_command

    def _patched_run_command(argv, **kw):
        argv = [
            "--enable-ldw-opt=true" if a == "--enable-ldw-opt=false" else a
            for a in argv
        ]
        return _orig_run_command(argv, **kw)

    _bu.run_command = _patched_run_command

BF16 = mybir.dt.float16
F32 = mybir.dt.float32
NPBF16 = np.float16

DIM = 512          # per-modality feature dim
N_TOK = 1024       # sequence length
NH = 4             # heads handled per core (one modality's heads)
D = 128            # head dim
NT = 8             # token tiles of 128
EPS = 1e-5
SCALE = 1.0 / float(np.sqrt(D))
VW = 132           # per-head v block width: 128 d + 1 ones + 3 pad
AF = mybir.ActivationFunctionType


def build_module(trivial: bool):
    """Build the per-core Bass program.  trivial=True assumes all LN gains are
    exactly 1 and biases exactly 0 (folded tables are plain cos/sin and the
    additive rope term vanishes); trivial=False uses full-width tables with
    g folded in and T3 pre-added into r_dram via a DRAM->DRAM copy."""
    nc = bacc.Bacc("TRN2", target_bir_lowering=False, debug=False, num_devices=8)

    xT = nc.dram_tensor("xT", [DIM, N_TOK], BF16, kind="ExternalInput")
    W = nc.dram_tensor("W", [DIM, 3 * DIM], BF16, kind="ExternalInput")
    if trivial:
        T1 = nc.dram_tensor("T1", [N_TOK, 64], BF16, kind="ExternalInput")
        T2N = nc.dram_tensor("T2N", [N_TOK, 64], BF16, kind="ExternalInput")
        T2P = nc.dram_tensor("T2P", [N_TOK, 64], BF16, kind="ExternalInput")
    else:
        T1 = nc.dram_tensor("T1", [N_TOK, 1024], BF16, kind="ExternalInput")
        T2 = nc.dram_tensor("T2", [N_TOK, 1024], BF16, kind="ExternalInput")
        T3 = nc.dram_tensor("T3", [N_TOK, 1024], BF16, kind="ExternalInput")
    out_d = nc.dram_tensor("out", [N_TOK, DIM], F32, kind="ExternalOutput")

    with tile.TileContext(nc) as tc, ExitStack() as ctx:
        consts = ctx.enter_context(tc.tile_pool(name="consts", bufs=1))
        small = ctx.enter_context(tc.tile_pool(name="small", bufs=4))
        upool = ctx.enter_context(tc.tile_pool(name="upool", bufs=2))
        rpool = ctx.enter_context(tc.tile_pool(name="rpool", bufs=2))
        epool = ctx.enter_context(tc.tile_pool(name="epool", bufs=2))
        dpool = ctx.enter_context(tc.tile_pool(name="dpool", bufs=1, space="DRAM"))
        psum_qk = ctx.enter_context(tc.tile_pool(name="psqk", bufs=2, space="PSUM"))
        psum_w = ctx.enter_context(tc.tile_pool(name="psw", bufs=2, space="PSUM"))
        psum_v = ctx.enter_context(tc.tile_pool(name="psv", bufs=2, space="PSUM"))

        # ---- input loads spread over three rings (sync/scalar HWDGE +
        # gpsimd SWDGE) so the first matmul's coarse ring-ordinal waits
        # cover as little as possible ----
        xr = xT.ap().rearrange("(a b) c -> b a c", b=128)
        wr = W.ap().rearrange("(a b) c -> b a c", b=128)
        xT_k, W_k = [], []
        for kc in range(4):
            xt = consts.tile([128, N_TOK], BF16, name=f"xT{kc}", tag=f"xT{kc}")
            xT_k.append(xt)
            wt = consts.tile([128, 3 * DIM], BF16, name=f"W{kc}", tag=f"W{kc}")
            W_k.append(wt)
        nc.sync.dma_start(out=xT_k[0], in_=xr[:, 0])
        nc.scalar.dma_start(out=W_k[0], in_=wr[:, 0])
        nc.sync.dma_start(out=xT_k[1], in_=xr[:, 1])
        nc.scalar.dma_start(out=W_k[1], in_=wr[:, 1])
        nc.gpsimd.dma_start(out=xT_k[2], in_=xr[:, 2])
        nc.gpsimd.dma_start(out=W_k[2], in_=wr[:, 2])
        nc.gpsimd.dma_start(out=xT_k[3], in_=xr[:, 3])
        nc.gpsimd.dma_start(out=W_k[3], in_=wr[:, 3])
        if trivial:
            cos_sb = consts.tile([128, NT, 64], BF16, tag="cos")
            nc.sync.dma_start(
                out=cos_sb, in_=T1.ap().rearrange("(a b) c -> b a c", b=128)
            )
            sinN_sb = consts.tile([128, NT, 64], BF16, tag="sinN")
            nc.sync.dma_start(
                out=sinN_sb, in_=T2N.ap().rearrange("(a b) c -> b a c", b=128)
            )
            sinP_sb = consts.tile([128, NT, 64], BF16, tag="sinP")
            nc.sync.dma_start(
                out=sinP_sb, in_=T2P.ap().rearrange("(a b) c -> b a c", b=128)
            )
        else:
            T1_sb = consts.tile([128, NT, 1024], BF16, tag="T1")
            nc.sync.dma_start(
                out=T1_sb, in_=T1.ap().rearrange("(a b) c -> b a c", b=128)
            )
            T2_sb = consts.tile([128, NT, 1024], BF16, tag="T2")
            nc.scalar.dma_start(
                out=T2_sb, in_=T2.ap().rearrange("(a b) c -> b a c", b=128)
            )
        eps_sb = consts.tile([128, 1], F32, tag="eps")
        nc.vector.memset(eps_sb, EPS)
        # preload the sqrt activation table while waiting for inputs
        warm = consts.tile([128, 1], F32, tag="warm")
        nc.scalar.activation(warm, eps_sb, AF.Sqrt)
        # warm the PE HAM clock gate (~4us of dummy matmuls) so the first
        # QKV matmuls run at 2.4GHz instead of the cold 1.2GHz
        dummy = consts.tile([128, 64], F32, tag="dummy")
        nc.vector.memset(dummy, 0.0)
        warm_ps = psum_w.tile([128, 512], F32, tag="w", name="warm_ps", bufs=2)
        for _ in range(36):
            nc.tensor.matmul(
                warm_ps[0:64, 0:64], lhsT=dummy, rhs=dummy,
                start=True, stop=True,
            )

        v_sb = consts.tile([128, NT, NH, VW], BF16, tag="v")
        nc.vector.memset(v_sb[:, :, :, 128:129], 1.0)

        qkT_sb = [
            [
                consts.tile(
                    [128, N_TOK], BF16, name=f"qkT{s}{h}", tag=f"qkT{s}{h}"
                )
                for h in range(NH)
            ]
            for s in range(2)
        ]
        out_sb = consts.tile([128, NT, DIM], F32, tag="osb")
        r_dram = dpool.tile([N_TOK, 2 * DIM], BF16, name="r_dram", tag="r_dram")
        if not trivial:
            # pre-add the additive rope/bias term; m1/m2 then accumulate on top
            nc.gpsimd.dma_start(out=r_dram, in_=T3.ap())

        def bcast(ap2d, dims):
            """[128, 64] AP -> [128, *dims, 64] with stride-0 broadcast dims."""
            p, last = ap2d.ap[0], ap2d.ap[-1]
            return bass.AP(
                tensor=ap2d.tensor,
                offset=ap2d.offset,
                ap=[p] + [[0, d] for d in dims] + [last],
            )

        def half(ap, i):
            return ap.rearrange("p (b half j) -> p b half j", half=2, j=64)[
                :, :, i, :
            ]

        # ---------------- stage A: one 128-token tile ----------------
        u_of = {}

        def stage_a1(t):
            qkv_ps = psum_qk.tile([128, 2 * DIM], F32, tag="qk", name="qkv_ps")
            v_ps = psum_v.tile([128, DIM], F32, tag="v", name="v_ps")
            for kc in range(4):
                for j in range(2):
                    nc.tensor.matmul(
                        qkv_ps[:, j * 512 : (j + 1) * 512],
                        lhsT=xT_k[kc][:, t * 128 : (t + 1) * 128],
                        rhs=W_k[kc][:, j * 512 : (j + 1) * 512],
                        start=(kc == 0),
                        stop=(kc == 3),
                    )
                nc.tensor.matmul(
                    v_ps,
                    lhsT=xT_k[kc][:, t * 128 : (t + 1) * 128],
                    rhs=W_k[kc][:, 1024:1536],
                    start=(kc == 0),
                    stop=(kc == 3),
                )

            u = upool.tile([128, 2 * DIM], BF16, tag="u", name="u")
            for s in range(2):
                st = small.tile([128, 6], F32, tag=f"st{s}", name="st")
                nc.vector.bn_stats(out=st, in_=qkv_ps[:, s * 512 : (s + 1) * 512])
                mv = small.tile([128, 2], F32, tag=f"mv{s}", name="mv")
                nc.vector.bn_aggr(out=mv, in_=st)
                sd = small.tile([128, 1], F32, tag=f"sd{s}", name="sd")
                nc.scalar.activation(sd, mv[:, 1:2], AF.Sqrt, bias=eps_sb)
                rstd = small.tile([128, 1], F32, tag=f"rs{s}", name="rstd")
                nc.vector.reciprocal(rstd, sd)
                nmr = small.tile([128, 1], F32, tag=f"nmr{s}", name="nmr")
                nc.vector.scalar_tensor_tensor(
                    out=nmr,
                    in0=mv[:, 0:1],
                    scalar=-1.0,
                    in1=rstd,
                    op0=mybir.AluOpType.mult,
                    op1=mybir.AluOpType.mult,
                )
                # u = q * rstd + (-mu * rstd), on ScalarE (frees VectorE)
                nc.scalar.activation(
                    out=u[:, s * 512 : (s + 1) * 512],
                    in_=qkv_ps[:, s * 512 : (s + 1) * 512],
                    func=AF.Identity,
                    scale=rstd,
                    bias=nmr,
                )

            # v (raw) into augmented per-head layout, on ScalarE
            nc.scalar.activation(
                out=v_sb[:, t, :, 0:128],
                in_=v_ps.rearrange("p (h d) -> p h d", h=NH),
                func=AF.Copy,
            )

            u_of[t] = u

        def stage_a2(t):
            u = u_of.pop(t)
            # rope: r = u * T1 + swap_half(u) * T2 (+ T3); the halves are
            # combined by accumulating SWDGE stores into r_dram (GpSimd is
            # otherwise idle and this frees VectorE of the add)
            m1 = rpool.tile([128, 2 * DIM], BF16, tag="m1", name="m1")
            m2 = rpool.tile([128, 2 * DIM], BF16, tag="m2", name="m2")
            if trivial:
                nc.vector.tensor_mul(
                    m1.rearrange("p (b j) -> p b j", j=64),
                    u.rearrange("p (b j) -> p b j", j=64),
                    bcast(cos_sb[:, t], (16,)),
                )
                nc.vector.tensor_mul(
                    half(m2, 0), half(u, 1), bcast(sinN_sb[:, t], (8,))
                )
                nc.vector.tensor_mul(
                    half(m2, 1), half(u, 0), bcast(sinP_sb[:, t], (8,))
                )
            else:
                nc.vector.tensor_mul(m1, u, T1_sb[:, t])
                nc.vector.tensor_mul(half(m2, 0), half(u, 1), half(T2_sb[:, t], 0))
                nc.vector.tensor_mul(half(m2, 1), half(u, 0), half(T2_sb[:, t], 1))
            rows = slice(t * 128, (t + 1) * 128)
            m1_op = (
                mybir.AluOpType.bypass if trivial else mybir.AluOpType.add
            )
            nc.gpsimd.dma_start(out=r_dram[rows, :], in_=m1, accum_op=m1_op)
            nc.gpsimd.dma_start(
                out=r_dram[rows, :], in_=m2, accum_op=mybir.AluOpType.add
            )

        def emit_transposes(H):
            rows = slice(H * 512, (H + 1) * 512)
            for h in range(NH):
                for s in range(2):
                    blk = (s * NH + h) * 128
                    nc.sync.dma_start(
                        out=qkT_sb[s][h][:, rows],
                        in_=r_dram[rows, blk : blk + 128],
                        transpose=True,
                    )

        # ---------------- stage B pieces ----------------
        # exp tiles: ets[h][kc] = exp(scores^T) [128 k, 1024 q] bf16
        ets = [[None] * NT for _ in range(NH)]

        def get_et(h, kc):
            # one slot per (head, k-block): 32 x 2KB/partition. No recycling
            # across heads, so wave-1 exps of late heads never wait on the
            # AV reads of early heads (which would deadlock the ACT queue).
            if ets[h][kc] is None:
                ets[h][kc] = epool.tile(
                    [128, N_TOK], BF16, tag=f"exp{h}_{kc}",
                    name=f"exp{h}_{kc}", bufs=1,
                )
            return ets[h][kc]

        def scores_half(h, kc, qh):
            """Quadrant scores: [128 k, 512 q] for (head, k-block, q-half).
            kc<4 + qh=0 only needs half-0 transposes, so it runs inside
            stage A's tail, moving ~11us of exp off the stage B span."""
            qT, kT = qkT_sb[0][h], qkT_sb[1][h]
            sc_w = psum_w.tile([128, 512], F32, tag="w", name="sc_w")
            nc.tensor.matmul(
                sc_w,
                lhsT=kT[:, kc * 128 : (kc + 1) * 128],
                rhs=qT[:, qh * 512 : (qh + 1) * 512],
                start=True,
                stop=True,
            )
            et = get_et(h, kc)
            nc.scalar.activation(
                out=et[:, qh * 512 : (qh + 1) * 512],
                in_=sc_w,
                func=AF.Exp,
                scale=SCALE,
            )

        def scores_full(h, kc):
            qT, kT = qkT_sb[0][h], qkT_sb[1][h]
            sc_ps = psum_qk.tile([128, 2 * DIM], F32, tag="qk", name="sc_ps")
            for qh in range(2):
                nc.tensor.matmul(
                    sc_ps[:, qh * 512 : (qh + 1) * 512],
                    lhsT=kT[:, kc * 128 : (kc + 1) * 128],
                    rhs=qT[:, qh * 512 : (qh + 1) * 512],
                    start=True,
                    stop=True,
                )
            et = get_et(h, kc)
            nc.scalar.activation(
                out=et,
                in_=sc_ps[:, 0:N_TOK],
                func=AF.Exp,
                scale=SCALE,
            )

        def emit_wave1():
            # (kc<4, qh=0) quadrants: ready as soon as half-0 transposes land
            for h in range(NH):
                for kc in range(4):
                    scores_half(h, kc, 0)

        def emit_scores_rest(h):
            for kc in range(4):
                scores_half(h, kc, 1)
            for kc in range(4, NT):
                scores_full(h, kc)

        def emit_av(h):
            for qc in range(NT):
                av = psum_v.tile([128, VW], F32, tag="v", name="av")
                for kc in range(NT):
                    et = ets[h][kc]
                    nc.tensor.matmul(
                        av[:, 0:129],
                        lhsT=et[:, qc * 128 : (qc + 1) * 128],
                        rhs=v_sb[:, kc, h, 0:129],
                        start=(kc == 0),
                        stop=(kc == NT - 1),
                    )
                rcp = small.tile([128, 1], F32, tag="rcp", name="rcp")
                nc.vector.reciprocal(rcp, av[:, 128:129])
                dst = out_sb[:, qc, h * 128 : (h + 1) * 128]
                nc.vector.tensor_scalar_mul(dst, av[:, 0:128], rcp)
                # per-(head, qc) output block on the SWDGE ring (idle in
                # stage B); tail after the last AV ~1us
                nc.gpsimd.dma_start(
                    out=out_d.ap()[
                        qc * 128 : (qc + 1) * 128, h * 128 : (h + 1) * 128
                    ],
                    in_=dst,
                )

        # ---------------- emission schedule ----------------
        stage_a1(0)
        for t in range(1, NT):
            stage_a1(t)
            stage_a2(t - 1)
            if t == 4:
                emit_transposes(0)
        stage_a2(NT - 1)
        emit_transposes(1)
        emit_wave1()

        emit_scores_rest(0)
        for h in range(NH):
            if h + 1 < NH:
                emit_scores_rest(h + 1)
            emit_av(h)

    nc.compile()
    return nc


def _rope_tables():
    inv_freq = 1.0 / (10000.0 ** (np.arange(0, D, 2, dtype=np.float32) / D))
    freqs = np.arange(N_TOK, dtype=np.float32)[:, None] * inv_freq[None, :]  # [n, 64]
    return np.cos(freqs), np.sin(freqs)


def _full_tables(g_q, b_q, g_k, b_k):
    """T1/T2/T3 [N_TOK, 1024] with LN gain/bias folded into the rope tables.
    Feature index layout matches u: (s, h, half, j)."""
    cos64, sin64 = _rope_tables()
    T1 = np.empty((N_TOK, 1024), np.float32)
    T2 = np.empty((N_TOK, 1024), np.float32)
    T3 = np.empty((N_TOK, 1024), np.float32)
    for s, (g, b) in enumerate(((g_q, b_q), (g_k, b_k))):
        g = g.reshape(NH, 2, 64)
        b = b.reshape(NH, 2, 64)
        for h in range(NH):
            base = s * 512 + h * 128
            lo, hi = slice(base, base + 64), slice(base + 64, base + 128)
            T1[:, lo] = g[h, 0] * cos64
            T1[:, hi] = g[h, 1] * cos64
            T2[:, lo] = -g[h, 1] * sin64
            T2[:, hi] = g[h, 0] * sin64
            T3[:, lo] = b[h, 0] * cos64 - b[h, 1] * sin64
            T3[:, hi] = b[h, 1] * cos64 + b[h, 0] * sin64
    return T1, T2, T3


def make_in_maps(x, a, Wqkv_x, Wqkv_a, g_qx, b_qx, g_kx, b_kx, g_qa, b_qa, g_ka, b_ka):
    """Returns (trivial, in_maps) for the 8 cores: core c = (batch c//2, modality c%2)."""
    x, a = np.asarray(x), np.asarray(a)
    Ws = (np.asarray(Wqkv_x), np.asarray(Wqkv_a))
    gb = (
        (np.asarray(g_qx), np.asarray(b_qx), np.asarray(g_kx), np.asarray(b_kx)),
        (np.asarray(g_qa), np.asarray(b_qa), np.asarray(g_ka), np.asarray(b_ka)),
    )
    trivial = all(
        np.all(g == 1.0) and np.all(b == 0.0)
        for (gq, bq, gk, bk) in gb
        for g, b in ((gq, bq), (gk, bk))
    )
    cos64, sin64 = _rope_tables()
    in_maps = []
    for c in range(8):
        i, m = c // 2, c % 2
        src = x[i] if m == 0 else a[i]
        im = {
            "xT": np.ascontiguousarray(src.T).astype(NPBF16),
            "W": Ws[m].astype(NPBF16),
        }
        if trivial:
            im["T1"] = cos64.astype(NPBF16)
            im["T2N"] = (-sin64).astype(NPBF16)
            im["T2P"] = sin64.astype(NPBF16)
        else:
            gq, bq, gk, bk = gb[m]
            T1, T2, T3 = _full_tables(gq, bq, gk, bk)
            im["T1"] = T1.astype(NPBF16)
            im["T2"] = T2.astype(NPBF16)
            im["T3"] = T3.astype(NPBF16)
        in_maps.append(im)
    return trivial, in_maps


_module_cache: dict[bool, object] = {}


def _get_module(trivial: bool):
    if trivial not in _module_cache:
        _module_cache[trivial] = build_module(trivial)
    return _module_cache[trivial]


def kernel(**inputs) -> np.ndarray:
    trivial, in_maps = make_in_maps(**inputs)
    nc = _get_module(trivial)
    res = run_bass_kernel_spmd(nc, in_maps, core_ids=list(range(8)))
    out = np.empty((4, N_TOK, 2 * DIM), np.float32)
    for c in range(8):
        i, m = c // 2, c % 2
        out[i, :, m * 512 : (m + 1) * 512] = res.results[c]["out"]
    return out
